# revision 26
# baseline (speedup 1.0000x reference)
"""Trainium2 Bass kernel for nn_Fast2Order_DE_Conv.

Math: out[b,o,ho,wo] = sum_{c,i,j} W[o, c*81+i*9+j] * p_i * p_j with
p_i = x[b, c, ho+di, wo+dj] (i = di*3+dj, 3x3 unfold of a 16-channel 64x64
image; output 62x62).

Algorithm (v3):
  * Diagonal terms (p_i^2) fold into a direct contraction over the unfold
    of x^2 (squared on the host, shipped as a second f16 input): two main
    matmul chunks of K=72 with weights Wd.
  * Off-diagonal terms use the squares basis (p_i+p_j)^2, 36 per channel,
    with the cross p_i^2 corrections folded into Wd.  Channels split
    7+7+2: groups A and B (63 x-unfold rows each) sit at PE row strips
    0-1 and 2-3 of one SBUF tile, so their selection matmuls (K=63,
    tile_position (0,0) / (64,0)) execute CONCURRENTLY pairwise; group C
    (K=18) runs serially.  Selection costs ~3N PE cycles instead of 6N.
  * Squares g = s^2 drain selection PSUM via ACT/DVE (f16), then 5 main
    matmul chunks of K=128 contract g.  Main total: 7 matmuls per tile.

Per 512-column spatial tile: 5 selection MMs (3 waves) + 7 main MMs
= ~10N PE cycles (vs 12N for the plain squares basis), and 2.5 instead of
3 merged square ops.

The 3x3 unfold is free: expressed in the DMA access pattern (overlapping
windows of the padded l' = ho*64+wo layout).  Pad columns (wo >= 62) may
hold garbage/NaN; every consumer view excludes them.

DMA: x loads ride gpsimd's software-DGE queue (16 engines) -- the
hardware-DGE rings share only ~3 DMA engines (~67 GB/s), exactly the
consumption rate, and starve the PE.  x^2's batch-0 half rides
sync/scalar HWDGE to spread issue cost.  HAM warmup: a burst of dummy
matmuls on a memset tile covers the ~9us DMA start-up window so the PE
clock gate stays at 2.4 GHz; a small tail burst keeps it warm through the
final drain.

Sharding: data-parallel over batch, 2 batches per core on 8 cores; W-side
constants are replicated.  Output gathered by simple concatenation.
"""

import functools

import numpy as np

import concourse.bacc as bacc
import concourse.mybir as mybir
from concourse.tile import TileContext
from concourse.bass_utils import run_bass_kernel_spmd

B, C, H, WIDTH = 16, 16, 64, 64
O = 128
HO = WO = 62
N_CORES = 8
B_LOC = B // N_CORES
OPAIRS = [(i, j) for i in range(9) for j in range(i + 1, 9)]  # 36 off-diag
ROW_TILES = [(0, 8), (8, 8), (16, 8), (24, 8), (32, 8), (40, 8), (48, 8), (56, 6)]
GC = 128
LFULL = HO * 64  # 3968 padded columns of l' = ho*64+wo
NG = 5  # g chunks: A0 A1 B0 B1 C (640 rows, 56+8 zero pad)


def _build_consts(Wf: np.ndarray):
    """W (128, 1296) -> (A_sel [128, 640] f16, W2T [640, 128] f16,
    Wd [144, 128] f16).

    g rows: A group (ch 0-6) F 0..251 pad 256, B (ch 7-13) F 256..507 pad
    512, C (ch 14,15) F 512..583 pad 640.  x-unfold rows: A/B at
    base + (di*3+dj)*7 + ch_local (bases 0 / 64 of one tile), C at
    (di*3+dj)*2 + ch_local of its own tile.  Wd rows pos*8 + cl per
    c-half, matching the x^2 unfold layout.
    """
    Wt = np.asarray(Wf, dtype=np.float64).reshape(O, C, 9, 9)
    Wsym = Wt + Wt.transpose(0, 1, 3, 2)
    A_sel = np.zeros((GC, 640), dtype=np.float32)
    W2T = np.zeros((640, O))
    for gi, (base, ch0, nch) in enumerate([(0, 0, 7), (64, 7, 7)]):
        for k in range(nch):
            c = ch0 + k
            for pf, (i, j) in enumerate(OPAIRS):
                F = gi * 256 + k * 36 + pf
                W2T[F] = 0.5 * Wsym[:, c, i, j]
                for pos in (i, j):
                    A_sel[base + pos * 7 + k, F] += 1.0
    for k, c in enumerate([14, 15]):
        for pf, (i, j) in enumerate(OPAIRS):
            F = 512 + k * 36 + pf
            W2T[F] = 0.5 * Wsym[:, c, i, j]
            for pos in (i, j):
                A_sel[pos * 2 + k, F] += 1.0
    Wd = np.zeros((2 * 72, O))
    for c in range(C):
        h, cl = divmod(c, 8)
        for pos in range(9):
            Wd[h * 72 + pos * 8 + cl] = (
                Wt[:, c, pos, pos]
                - 0.5 * (Wsym[:, c, pos, :].sum(-1) - 2.0 * Wt[:, c, pos, pos])
            )
    return (
        A_sel.astype(np.float16),
        np.ascontiguousarray(W2T).astype(np.float16),
        np.ascontiguousarray(Wd).astype(np.float16),
    )


def _unfold_ap(x_d, b: int, c0: int, nch: int, di: int, lt_load: int):
    """Source AP for one di of an unfold load: dims (dj, ch, l) matching a
    contiguous target partition run ordered dj*nch + ch."""
    ap = x_d[b, c0 : c0 + nch, di, 0:3].unsqueeze(-1)
    v = ap.ap
    v[0] = [1, 3]
    v[1] = [H * WIDTH, nch]
    v[2] = [1, lt_load]
    return ap


def build_nc(reps: int = 1, skew: int = 1, warmups: int = 36, tailwarm: int = 5):
    """Build the per-core program.  reps>1 wraps the compute body in an
    on-chip loop (loads stay outside; loop timing only)."""
    f32, f16 = mybir.dt.float32, mybir.dt.float16
    nc = bacc.Bacc("TRN2", target_bir_lowering=False)
    x_d = nc.dram_tensor("x_loc", [B_LOC, C, H, WIDTH], f16, kind="ExternalInput")
    x2_d = nc.dram_tensor("x2_loc", [B_LOC, C, H, WIDTH], f16, kind="ExternalInput")
    a_d = nc.dram_tensor("aselT", [GC, NG * GC], f16, kind="ExternalInput")
    w_d = nc.dram_tensor("w2T", [NG * GC, O], f16, kind="ExternalInput")
    wd_d = nc.dram_tensor("wd", [2 * 72, O], f16, kind="ExternalInput")
    o_d = nc.dram_tensor("out_loc", [B_LOC, O, HO, WO], f32, kind="ExternalOutput")

    with TileContext(nc) as tc:
        with (
            tc.tile_pool(name="const", bufs=1) as cpool,
            tc.tile_pool(name="xin", bufs=B_LOC) as xpool,
            tc.tile_pool(name="gbuf", bufs=3 * (skew + 2)) as gpool,
            tc.tile_pool(name="tmpbuf", bufs=4) as tmppool,
            tc.tile_pool(name="obuf", bufs=6) as opool,
            tc.tile_pool(name="ps_ab", bufs=2, space="PSUM") as pabpool,
            tc.tile_pool(name="ps_c", bufs=2, space="PSUM") as pcpool,
            tc.tile_pool(name="ps_out", bufs=2, space="PSUM") as popool,
        ):
            # warmup tile first: memset has no DMA dependency, so dummy
            # matmuls start immediately and hold the HAM clock gate open
            # (2.4 GHz) through the ~9us DMA-engine start-up window
            wt = cpool.tile([GC, 512], f16, tag="warm")
            nc.vector.memset(wt[:], 0.0)

            a_r = cpool.tile([GC, NG * GC], f16, tag="a_r")
            nc.sync.dma_start(a_r[:], a_d[:])

            def load_sel(xab_t, xc_t, b, eng):
                """Unfold loads for the selection groups of batch b."""
                for di in range(3):
                    hi = min(LFULL, H * WIDTH - di * 64 - 2)
                    for base, c0 in ((0, 0), (64, 7)):
                        eng.dma_start(
                            xab_t[base + di * 21 : base + di * 21 + 21, 0:hi],
                            _unfold_ap(x_d, b, c0, 7, di, hi),
                        )
                    eng.dma_start(
                        xc_t[di * 6 : di * 6 + 6, 0:hi],
                        _unfold_ap(x_d, b, 14, 2, di, hi),
                    )

            def load_x2(x2_t, b, h, col0, col1, eng):
                """x^2 unfold load for c-half h of batch b."""
                for di in range(3):
                    hi = min(col1, H * WIDTH - di * 64 - 2)
                    if hi > col0:
                        ap = _unfold_ap(x2_d, b, h * 8, 8, di, hi - col0)
                        ap.offset += col0
                        eng.dma_start(x2_t[di * 24 : di * 24 + 24, col0:hi], ap)

            xr_all = []
            for b in range(B_LOC):
                xab_t = xpool.tile([GC, LFULL], f16, tag="xab", name=f"xab_{b}")
                xc_t = xpool.tile([18, LFULL], f16, tag="xc", name=f"xc_{b}")
                x2_t = [
                    xpool.tile([72, LFULL], f16, tag=f"x2{h}", name=f"x2{h}_{b}")
                    for h in range(2)
                ]
                xr_all.append((xab_t, xc_t, x2_t))

            # batch-0 selection x via SWDGE (gpsimd), x^2 batch 0 via the
            # two HWDGE queues (column-split so tile-0 columns land first),
            # then batch 1 entirely via SWDGE, then weights
            load_sel(xr_all[0][0], xr_all[0][1], 0, nc.gpsimd)
            load_x2(xr_all[0][2][0], 0, 0, 0, 1024, nc.sync)
            load_x2(xr_all[0][2][1], 0, 1, 0, 1024, nc.scalar)
            for b in range(1, B_LOC):
                load_sel(xr_all[b][0], xr_all[b][1], b, nc.gpsimd)
                for h in range(2):
                    load_x2(xr_all[b][2][h], b, h, 0, LFULL, nc.gpsimd)
            load_x2(xr_all[0][2][0], 0, 0, 1024, LFULL, nc.sync)
            load_x2(xr_all[0][2][1], 0, 1, 1024, LFULL, nc.scalar)
            w_r = cpool.tile([GC, NG, O], f16, tag="w_r")
            nc.sync.dma_start(w_r[:], w_d[:].rearrange("(k p) o -> p k o", p=GC))
            wd_r = cpool.tile([72, 2, O], f16, tag="wd_r")
            nc.sync.dma_start(wd_r[:], wd_d[:].rearrange("(h p) o -> p h o", p=72))

            # greedy ACT/DVE balancing for PSUM-draining elementwise ops
            # (measured us per [*,1024-col] op: ACT square 1.05, DVE
            # square = f16 cast + f16 mul 1.86, copies ~0.65)
            eng_busy = {"act": 0.0, "dve": 0.0}

            def square(g_t, ps_s, lt, halves=2):
                scale = halves / 2.0
                gv = g_t[:, :halves, :lt]
                pv = ps_s[:, :halves, :lt]
                if eng_busy["act"] + 1.05 * scale <= eng_busy["dve"] + 1.86 * scale:
                    nc.scalar.square(gv, pv)
                    eng_busy["act"] += 1.05 * scale
                else:
                    tmp = tmppool.tile([GC, 2, 512], f16, tag="sq_tmp")
                    tv = tmp[:, :halves, :lt]
                    nc.vector.tensor_copy(tv, pv)
                    nc.vector.tensor_mul(gv, tv, tv)
                    eng_busy["dve"] += 1.86 * scale

            def out_copy(o_view, ps_view):
                if eng_busy["act"] + 0.68 < eng_busy["dve"] + 0.62:
                    nc.scalar.copy(o_view, ps_view)
                    eng_busy["act"] += 0.68
                else:
                    nc.vector.tensor_copy(o_view, ps_view)
                    eng_busy["dve"] += 0.62

            def do_mains(st):
                """Main matmuls + drain for a tile whose squares are issued."""
                b, ho0, nr, g_ts = st
                lt = nr * 64
                c0 = ho0 * 64
                x2_t = xr_all[b][2]
                ps_o = popool.tile([O, 512], f32, tag="ps_o", name="ps_o")
                for h in range(2):
                    nc.tensor.matmul(
                        ps_o[:, :lt],
                        wd_r[:, h, :],
                        x2_t[h][:, c0 : c0 + lt],
                        start=(h == 0),
                        stop=False,
                    )
                for kk in range(NG):
                    gt, half = g_ts[kk % 2 if kk < 4 else 2], (kk // 2 if kk < 4 else 0)
                    nc.tensor.matmul(
                        ps_o[:, :lt],
                        w_r[:, kk, :],
                        gt[:, half, :lt],
                        start=False,
                        stop=(kk == NG - 1),
                    )
                # compact to [O, nr*62] so the store uses contiguous chunks
                o_t = opool.tile([O, 8 * WO], f32, tag="o", name="o_t")
                ps_view = ps_o[:, :lt].rearrange("o (r w) -> o r w", w=64)
                o_view = o_t[:, : nr * WO].rearrange("o (r w) -> o r w", w=WO)
                out_copy(o_view, ps_view[:, :, :WO])
                nc.gpsimd.dma_start(
                    o_d[b, :, ho0 : ho0 + nr, :],
                    o_t[:, : nr * WO],
                )

            def warmup(n):
                for _ in range(n):
                    ps_w = popool.tile([O, 512], f32, tag="ps_o", name="warm")
                    nc.tensor.matmul(
                        ps_w[:], wt[:, :GC], wt[:], start=True, stop=True
                    )

            def body(it=None, unroll=1):
                pending = []
                for b in range(B_LOC):
                    xab_t, xc_t, _ = xr_all[b]
                    for ho0, nr in ROW_TILES:
                        lt = nr * 64
                        c0 = ho0 * 64
                        ps_ab = [
                            pabpool.tile([GC, 2, 512], f32, tag="ps_ab", name="ps_ab")
                            for _ in range(2)
                        ]
                        ps_c = pcpool.tile([GC, 1, 512], f32, tag="ps_c", name="ps_c")
                        g_ts = [
                            gpool.tile([GC, 2, 512], f16, tag="g", name="g_t")
                            for _ in range(2)
                        ] + [gpool.tile([GC, 1, 512], f16, tag="gc", name="gc_t")]
                        # wave 1: A0 (strips 0-1) || B0 (strips 2-3)
                        for wv in range(2):
                            nc.tensor.matmul(
                                ps_ab[wv][:, 0, :lt],
                                a_r[0:63, wv * GC : wv * GC + GC],
                                xab_t[0:63, c0 : c0 + lt],
                                start=True, stop=True, tile_position=(0, 0),
                            )
                            nc.tensor.matmul(
                                ps_ab[wv][:, 1, :lt],
                                a_r[64:127, (2 + wv) * GC : (3 + wv) * GC],
                                xab_t[64:127, c0 : c0 + lt],
                                start=True, stop=True, tile_position=(64, 0),
                            )
                            square(g_ts[wv], ps_ab[wv], lt)
                        nc.tensor.matmul(
                            ps_c[:, 0, :lt],
                            a_r[0:18, 4 * GC : 5 * GC],
                            xc_t[0:18, c0 : c0 + lt],
                            start=True, stop=True, tile_position=(0, 0),
                        )
                        square(g_ts[2], ps_c, lt, halves=1)
                        pending.append((b, ho0, nr, g_ts))
                        if len(pending) > skew:
                            do_mains(pending.pop(0))
                # keep the clock gate warm through the elementwise drain of
                # the last tile, then finish its mains
                warmup(tailwarm)
                for st in pending:
                    do_mains(st)

            warmup(warmups)
            if reps == 1:
                body()
            else:
                hint = (
                    mybir.EngineType.PE,
                    mybir.EngineType.Activation,
                    mybir.EngineType.DVE,
                    mybir.EngineType.SP,
                    mybir.EngineType.Pool,
                )
                with tc.For_i(0, reps, 1, hint_engines=hint) as _it:
                    body()
    nc.compile()
    return nc


@functools.lru_cache(maxsize=1)
def _cached_nc():
    return build_nc()


def _core_inputs(x: np.ndarray, consts, k: int) -> dict:
    A_sel, W2T, Wd = consts
    x_r = np.asarray(x, dtype=np.float32).astype(np.float16)
    x2_r = (x_r.astype(np.float32) ** 2).astype(np.float16)
    return {
        "x_loc": np.ascontiguousarray(x_r[k * B_LOC : (k + 1) * B_LOC]),
        "x2_loc": np.ascontiguousarray(x2_r[k * B_LOC : (k + 1) * B_LOC]),
        "aselT": A_sel,
        "w2T": W2T,
        "wd": Wd,
    }


def kernel(x: np.ndarray, W: np.ndarray, _trace: bool = False):
    x = np.asarray(x, dtype=np.float32)
    W = np.asarray(W, dtype=np.float32)
    consts = _build_consts(W)

    nc = _cached_nc()
    in_maps = [_core_inputs(x, consts, k) for k in range(N_CORES)]
    try:
        r = run_bass_kernel_spmd(
            nc, in_maps, core_ids=list(range(N_CORES)), trace=_trace
        )
    except Exception:
        # transient NRT_EXEC_UNIT_UNRECOVERABLE has been observed once on
        # this fabric; a fresh attempt recovers
        r = run_bass_kernel_spmd(
            nc, in_maps, core_ids=list(range(N_CORES)), trace=_trace
        )
    out = np.concatenate([m["out_loc"] for m in r.results], axis=0)
    if _trace:
        kernel.last_result = r
    return out


if __name__ == "__main__":
    rng = np.random.default_rng(0)
    x = rng.standard_normal((B, C, H, WIDTH), dtype=np.float32)
    W = rng.standard_normal((O, C * 81), dtype=np.float32)
    out = kernel(x, W)
    print("out shape", out.shape, out.dtype)


# revision 29
# speedup vs baseline: 1.2093x; 1.2093x over previous
"""Trainium2 Bass kernel for nn_Fast2Order_DE_Conv.

Math: out[b,o,ho,wo] = sum_{c,i,j} W[o, c*81+i*9+j] * p_i * p_j with
p_i = x[b, c, ho+di, wo+dj] (i = di*3+dj, 3x3 unfold of a 16-channel 64x64
image; output 62x62).

Algorithm (v3):
  * Diagonal terms (p_i^2) fold into a direct contraction over the unfold
    of x^2 (squared on the host, shipped as a second f16 input): two main
    matmul chunks of K=72 with weights Wd.
  * Off-diagonal terms use the squares basis (p_i+p_j)^2, 36 per channel,
    with the cross p_i^2 corrections folded into Wd.  Channels split
    7+7+2: groups A and B (63 x-unfold rows each) sit at PE row strips
    0-1 and 2-3 of one SBUF tile, so their selection matmuls (K=63,
    tile_position (0,0) / (64,0)) execute CONCURRENTLY pairwise; group C
    (K=18) runs serially.  Selection costs ~3N PE cycles instead of 6N.
  * Squares g = s^2 drain selection PSUM via ACT/DVE (f16), then 5 main
    matmul chunks of K=128 contract g.  Main total: 7 matmuls per tile.

Per 512-column spatial tile: 5 selection MMs (3 waves) + 7 main MMs
= ~10N PE cycles (vs 12N for the plain squares basis), and 2.5 instead of
3 merged square ops.

The 3x3 unfold is free: expressed in the DMA access pattern (overlapping
windows of the padded l' = ho*64+wo layout).  Pad columns (wo >= 62) may
hold garbage/NaN; every consumer view excludes them.

DMA: x loads ride gpsimd's software-DGE queue (16 engines) -- the
hardware-DGE rings share only ~3 DMA engines (~67 GB/s), exactly the
consumption rate, and starve the PE.  x^2's batch-0 half rides
sync/scalar HWDGE to spread issue cost.  HAM warmup: a burst of dummy
matmuls on a memset tile covers the ~9us DMA start-up window so the PE
clock gate stays at 2.4 GHz; a small tail burst keeps it warm through the
final drain.

Sharding: data-parallel over batch, 2 batches per core on 8 cores; W-side
constants are replicated.  Output gathered by simple concatenation.
"""

import functools

import numpy as np

import concourse.bacc as bacc
import concourse.mybir as mybir
from concourse.tile import TileContext
from concourse.bass_utils import run_bass_kernel_spmd

B, C, H, WIDTH = 16, 16, 64, 64
O = 128
HO = WO = 62
N_CORES = 8
B_LOC = B // N_CORES
OPAIRS = [(i, j) for i in range(9) for j in range(i + 1, 9)]  # 36 off-diag
ROW_TILES = [(0, 8), (8, 8), (16, 8), (24, 8), (32, 8), (40, 8), (48, 8), (56, 6)]
GC = 128
LFULL = HO * 64  # 3968 padded columns of l' = ho*64+wo
NG = 5  # g chunks: A0 A1 B0 B1 C (640 rows, 56+8 zero pad)


def _build_consts(Wf: np.ndarray):
    """W (128, 1296) -> (A_sel [128, 640] f16, W2T [640, 128] f16,
    Wd [144, 128] f16).

    g rows: A group (ch 0-6) F 0..251 pad 256, B (ch 7-13) F 256..507 pad
    512, C (ch 14,15) F 512..583 pad 640.  x-unfold rows: A/B at
    base + (di*3+dj)*7 + ch_local (bases 0 / 64 of one tile), C at
    (di*3+dj)*2 + ch_local of its own tile.  Wd rows pos*8 + cl per
    c-half, matching the x^2 unfold layout.
    """
    Wt = np.asarray(Wf, dtype=np.float64).reshape(O, C, 9, 9)
    Wsym = Wt + Wt.transpose(0, 1, 3, 2)
    A_sel = np.zeros((GC, 640), dtype=np.float32)
    W2T = np.zeros((640, O))
    for gi, (base, ch0, nch) in enumerate([(0, 0, 7), (64, 7, 7)]):
        for k in range(nch):
            c = ch0 + k
            for pf, (i, j) in enumerate(OPAIRS):
                F = gi * 256 + k * 36 + pf
                W2T[F] = 0.5 * Wsym[:, c, i, j]
                for pos in (i, j):
                    A_sel[base + pos * 7 + k, F] += 1.0
    for k, c in enumerate([14, 15]):
        for pf, (i, j) in enumerate(OPAIRS):
            F = 512 + k * 36 + pf
            W2T[F] = 0.5 * Wsym[:, c, i, j]
            for pos in (i, j):
                A_sel[pos * 2 + k, F] += 1.0
    Wd = np.zeros((2 * 72, O))
    for c in range(C):
        h, cl = divmod(c, 8)
        for pos in range(9):
            Wd[h * 72 + pos * 8 + cl] = (
                Wt[:, c, pos, pos]
                - 0.5 * (Wsym[:, c, pos, :].sum(-1) - 2.0 * Wt[:, c, pos, pos])
            )
    return (
        A_sel.astype(np.float16),
        np.ascontiguousarray(W2T).astype(np.float16),
        np.ascontiguousarray(Wd).astype(np.float16),
    )


def _unfold_ap(x_d, b: int, c0: int, nch: int, di: int, lt_load: int):
    """Source AP for one di of an unfold load: dims (dj, ch, l) matching a
    contiguous target partition run ordered dj*nch + ch."""
    ap = x_d[b, c0 : c0 + nch, di, 0:3].unsqueeze(-1)
    v = ap.ap
    v[0] = [1, 3]
    v[1] = [H * WIDTH, nch]
    v[2] = [1, lt_load]
    return ap


def build_nc(reps: int = 1, skew: int = 1, warmups: int = 44, tailwarm: int = 5):
    """Build the per-core program.  reps>1 wraps the compute body in an
    on-chip loop (loads stay outside; loop timing only)."""
    f32, f16 = mybir.dt.float32, mybir.dt.float16
    nc = bacc.Bacc("TRN2", target_bir_lowering=False)
    x_d = nc.dram_tensor("x_loc", [B_LOC, C, H, WIDTH], f16, kind="ExternalInput")
    x2_d = nc.dram_tensor("x2_loc", [B_LOC, C, H, WIDTH], f16, kind="ExternalInput")
    a_d = nc.dram_tensor("aselT", [GC, NG * GC], f16, kind="ExternalInput")
    w_d = nc.dram_tensor("w2T", [NG * GC, O], f16, kind="ExternalInput")
    wd_d = nc.dram_tensor("wd", [2 * 72, O], f16, kind="ExternalInput")
    o_d = nc.dram_tensor("out_loc", [B_LOC, O, HO, WO], f32, kind="ExternalOutput")

    with TileContext(nc) as tc:
        with (
            tc.tile_pool(name="const", bufs=1) as cpool,
            tc.tile_pool(name="xin", bufs=B_LOC) as xpool,
            tc.tile_pool(name="gbuf", bufs=3 * (skew + 2)) as gpool,
            tc.tile_pool(name="tmpbuf", bufs=4) as tmppool,
            tc.tile_pool(name="obuf", bufs=6) as opool,
            tc.tile_pool(name="ps_ab", bufs=2, space="PSUM") as pabpool,
            tc.tile_pool(name="ps_c", bufs=2, space="PSUM") as pcpool,
            tc.tile_pool(name="ps_out", bufs=2, space="PSUM") as popool,
        ):
            # warmup tile first: memset has no DMA dependency, so dummy
            # matmuls start immediately and hold the HAM clock gate open
            # (2.4 GHz) through the ~9us DMA-engine start-up window
            wt = cpool.tile([GC, 512], f16, tag="warm")
            nc.vector.memset(wt[:], 0.0)

            a_r = cpool.tile([GC, NG * GC], f16, tag="a_r")
            nc.sync.dma_start(a_r[:], a_d[:])

            def load_sel(xab_t, xc_t, b, eng):
                """Unfold loads for the selection groups of batch b."""
                for di in range(3):
                    hi = min(LFULL, H * WIDTH - di * 64 - 2)
                    for base, c0 in ((0, 0), (64, 7)):
                        eng.dma_start(
                            xab_t[base + di * 21 : base + di * 21 + 21, 0:hi],
                            _unfold_ap(x_d, b, c0, 7, di, hi),
                        )
                    eng.dma_start(
                        xc_t[di * 6 : di * 6 + 6, 0:hi],
                        _unfold_ap(x_d, b, 14, 2, di, hi),
                    )

            def load_x2(x2_t, b, h, col0, col1, eng):
                """x^2 unfold load for c-half h of batch b."""
                for di in range(3):
                    hi = min(col1, H * WIDTH - di * 64 - 2)
                    if hi > col0:
                        ap = _unfold_ap(x2_d, b, h * 8, 8, di, hi - col0)
                        ap.offset += col0
                        eng.dma_start(x2_t[di * 24 : di * 24 + 24, col0:hi], ap)

            xr_all = []
            for b in range(B_LOC):
                xab_t = xpool.tile([GC, LFULL], f16, tag="xab", name=f"xab_{b}")
                xc_t = xpool.tile([18, LFULL], f16, tag="xc", name=f"xc_{b}")
                x2_t = [
                    xpool.tile([72, LFULL], f16, tag=f"x2{h}", name=f"x2{h}_{b}")
                    for h in range(2)
                ]
                xr_all.append((xab_t, xc_t, x2_t))

            # all x-side loads via SWDGE (gpsimd) -- the HWDGE rings top out
            # at ~45-67 GB/s total, below the consumption rate.  sync only
            # carries the small constants (and half the output stores, to
            # keep the gpsimd issue queue short).
            for b in range(B_LOC):
                load_sel(xr_all[b][0], xr_all[b][1], b, nc.gpsimd)
                for h in range(2):
                    load_x2(xr_all[b][2][h], b, h, 0, LFULL, nc.gpsimd)
            w_r = cpool.tile([GC, NG, O], f16, tag="w_r")
            nc.sync.dma_start(w_r[:], w_d[:].rearrange("(k p) o -> p k o", p=GC))
            wd_r = cpool.tile([72, 2, O], f16, tag="wd_r")
            nc.sync.dma_start(wd_r[:], wd_d[:].rearrange("(h p) o -> p h o", p=72))

            # greedy ACT/DVE balancing for PSUM-draining elementwise ops
            # (measured us per [*,1024-col] op: ACT square 1.05, DVE
            # square = f16 cast + f16 mul 1.86, copies ~0.65)
            eng_busy = {"act": 0.0, "dve": 0.0}

            def square(g_t, ps_s, lt, halves=2):
                scale = halves / 2.0
                gv = g_t[:, :halves, :lt]
                pv = ps_s[:, :halves, :lt]
                if eng_busy["act"] + 1.05 * scale <= eng_busy["dve"] + 1.86 * scale:
                    nc.scalar.square(gv, pv)
                    eng_busy["act"] += 1.05 * scale
                else:
                    tmp = tmppool.tile([GC, 2, 512], f16, tag="sq_tmp")
                    tv = tmp[:, :halves, :lt]
                    nc.vector.tensor_copy(tv, pv)
                    nc.vector.tensor_mul(gv, tv, tv)
                    eng_busy["dve"] += 1.86 * scale

            def out_copy(o_view, ps_view):
                if eng_busy["act"] + 0.68 < eng_busy["dve"] + 0.62:
                    nc.scalar.copy(o_view, ps_view)
                    eng_busy["act"] += 0.68
                else:
                    nc.vector.tensor_copy(o_view, ps_view)
                    eng_busy["dve"] += 0.62

            def do_mains(st):
                """Main matmuls + drain for a tile whose squares are issued."""
                b, ho0, nr, g_ts = st
                lt = nr * 64
                c0 = ho0 * 64
                x2_t = xr_all[b][2]
                ps_o = popool.tile([O, 512], f32, tag="ps_o", name="ps_o")
                for h in range(2):
                    nc.tensor.matmul(
                        ps_o[:, :lt],
                        wd_r[:, h, :],
                        x2_t[h][:, c0 : c0 + lt],
                        start=(h == 0),
                        stop=False,
                    )
                for kk in range(NG):
                    gt, half = g_ts[kk % 2 if kk < 4 else 2], (kk // 2 if kk < 4 else 0)
                    nc.tensor.matmul(
                        ps_o[:, :lt],
                        w_r[:, kk, :],
                        gt[:, half, :lt],
                        start=False,
                        stop=(kk == NG - 1),
                    )
                # compact to [O, nr*62] so the store uses contiguous chunks
                o_t = opool.tile([O, 8 * WO], f32, tag="o", name="o_t")
                ps_view = ps_o[:, :lt].rearrange("o (r w) -> o r w", w=64)
                o_view = o_t[:, : nr * WO].rearrange("o (r w) -> o r w", w=WO)
                out_copy(o_view, ps_view[:, :, :WO])
                seng = nc.gpsimd if (ho0 // 8) % 2 == 0 else nc.sync
                seng.dma_start(
                    o_d[b, :, ho0 : ho0 + nr, :],
                    o_t[:, : nr * WO],
                )

            def warmup(n):
                for _ in range(n):
                    ps_w = popool.tile([O, 512], f32, tag="ps_o", name="warm")
                    nc.tensor.matmul(
                        ps_w[:], wt[:, :GC], wt[:], start=True, stop=True
                    )

            def body(it=None, unroll=1):
                pending = []
                for b in range(B_LOC):
                    xab_t, xc_t, _ = xr_all[b]
                    for ho0, nr in ROW_TILES:
                        lt = nr * 64
                        c0 = ho0 * 64
                        ps_ab = [
                            pabpool.tile([GC, 2, 512], f32, tag="ps_ab", name="ps_ab")
                            for _ in range(2)
                        ]
                        ps_c = pcpool.tile([GC, 1, 512], f32, tag="ps_c", name="ps_c")
                        g_ts = [
                            gpool.tile([GC, 2, 512], f16, tag="g", name="g_t")
                            for _ in range(2)
                        ] + [gpool.tile([GC, 1, 512], f16, tag="gc", name="gc_t")]
                        # wave 1: A0 (strips 0-1) || B0 (strips 2-3)
                        for wv in range(2):
                            nc.tensor.matmul(
                                ps_ab[wv][:, 0, :lt],
                                a_r[0:63, wv * GC : wv * GC + GC],
                                xab_t[0:63, c0 : c0 + lt],
                                start=True, stop=True, tile_position=(0, 0),
                            )
                            nc.tensor.matmul(
                                ps_ab[wv][:, 1, :lt],
                                a_r[64:127, (2 + wv) * GC : (3 + wv) * GC],
                                xab_t[64:127, c0 : c0 + lt],
                                start=True, stop=True, tile_position=(64, 0),
                            )
                            square(g_ts[wv], ps_ab[wv], lt)
                        nc.tensor.matmul(
                            ps_c[:, 0, :lt],
                            a_r[0:18, 4 * GC : 5 * GC],
                            xc_t[0:18, c0 : c0 + lt],
                            start=True, stop=True, tile_position=(0, 0),
                        )
                        square(g_ts[2], ps_c, lt, halves=1)
                        pending.append((b, ho0, nr, g_ts))
                        if len(pending) > skew:
                            do_mains(pending.pop(0))
                # keep the clock gate warm through the elementwise drain of
                # the last tile, then finish its mains
                warmup(tailwarm)
                for st in pending:
                    do_mains(st)

            warmup(warmups)
            if reps == 1:
                body()
            else:
                hint = (
                    mybir.EngineType.PE,
                    mybir.EngineType.Activation,
                    mybir.EngineType.DVE,
                    mybir.EngineType.SP,
                    mybir.EngineType.Pool,
                )
                with tc.For_i(0, reps, 1, hint_engines=hint) as _it:
                    body()
    nc.compile()
    return nc


@functools.lru_cache(maxsize=1)
def _cached_nc():
    return build_nc()


def _core_inputs(x: np.ndarray, consts, k: int) -> dict:
    A_sel, W2T, Wd = consts
    x_r = np.asarray(x, dtype=np.float32).astype(np.float16)
    x2_r = (x_r.astype(np.float32) ** 2).astype(np.float16)
    return {
        "x_loc": np.ascontiguousarray(x_r[k * B_LOC : (k + 1) * B_LOC]),
        "x2_loc": np.ascontiguousarray(x2_r[k * B_LOC : (k + 1) * B_LOC]),
        "aselT": A_sel,
        "w2T": W2T,
        "wd": Wd,
    }


def kernel(x: np.ndarray, W: np.ndarray, _trace: bool = False):
    x = np.asarray(x, dtype=np.float32)
    W = np.asarray(W, dtype=np.float32)
    consts = _build_consts(W)

    nc = _cached_nc()
    in_maps = [_core_inputs(x, consts, k) for k in range(N_CORES)]
    try:
        r = run_bass_kernel_spmd(
            nc, in_maps, core_ids=list(range(N_CORES)), trace=_trace
        )
    except Exception:
        # transient NRT_EXEC_UNIT_UNRECOVERABLE has been observed once on
        # this fabric; a fresh attempt recovers
        r = run_bass_kernel_spmd(
            nc, in_maps, core_ids=list(range(N_CORES)), trace=_trace
        )
    out = np.concatenate([m["out_loc"] for m in r.results], axis=0)
    if _trace:
        kernel.last_result = r
    return out


if __name__ == "__main__":
    rng = np.random.default_rng(0)
    x = rng.standard_normal((B, C, H, WIDTH), dtype=np.float32)
    W = rng.standard_normal((O, C * 81), dtype=np.float32)
    out = kernel(x, W)
    print("out shape", out.shape, out.dtype)
